# revision 1
# baseline (speedup 1.0000x reference)
"""Trainium2 Bass kernel for nn_DeepSCTransformerBlock.

Sharding: 8 cores = 4 batches x 2 branches (gene/expr). Zero collectives.
One SPMD program; the gene branch is expressed through the expr skeleton by
feeding fused_w = [[0],[I]] so Qf/Kf reduce to the gene projections, and the
(s2, d1) normalization coefficients (cA, cB) are per-core input data:
  gene: scale_i = 1/(1*s2 + 1e-8*d1)   [signed-L1 renorm of softmax*mask]
  expr: scale_i = 1/(0*s2 + 1*d1)      [plain softmax denominator]
where E = exp(S), P = E*M, d1 = colsum(E), s2 = colsum(P); O = scale * (P @ V).

Layout: activations flow feature-major ([d, tokens]) through matmul chains;
the residual stream and softmax statistics are token-major. S is computed
transposed ([j, i]) so the AV contraction needs no transposes of the attention
matrix; s2 falls out of a ones-column appended to V. Matmuls run in float32r
(fp32 bits, 1.5 cycles/row on the PE).
"""

import numpy as np

B, G, D, H = 4, 1024, 512, 8
HD = D // H
DFF = 4 * D
P = 128
ICH = G // P      # 8 token chunks
KC = D // P       # 4 feature chunks
FC = DFF // P     # 16 ffn-hidden chunks
NIH = 2           # i halves (free-dim 512 per matmul)
IH = G // NIH     # 512
SCALE = 1.0 / (HD ** 0.5)
EPS = 1e-5

_cache = {}


def _build_program(split_waits=True):
    import contextlib
    import concourse.bass as bass
    import concourse.mybir as mybir
    import concourse.tile as tile
    from concourse.masks import make_identity

    # walrus CoreV3 codegen rejects instructions carrying >1 sem wait at the
    # Tile end-of-kernel drain; split the waits across single-wait nops.
    def _patched_drain_and_barrier(self, tick_clock, wait_clock):
        nc = self.nc
        drain_inst = nc.sync.drain()
        wait_clock.add_sem_waits(
            drain_inst.ins, tile.ScopedClock({None: tick_clock.global_clock})
        )
        si = drain_inst.ins.sync_info
        if si is not None and si.on_wait and len(si.on_wait) > 1:
            waits = list(si.on_wait)
            si.on_wait = waits[:1]
            for i in range(1, len(waits)):
                nop = nc.sync.nop(hint="drain_wait_split", nofuse=True)
                nop.ins.sync_info = mybir.SyncInfo(
                    on_wait=waits[i : i + 1], on_update=[]
                )
        nc.all_engine_barrier()
        assert self.sems is not None
        popped = nc._tile_sem_poison_stack.pop()
        assert popped is self._sem_poison
        nc.clear_and_free_semaphores(list(self.sems.allocated().values()))
        nc.all_engine_barrier()

    tile.TileContext._drain_and_barrier = _patched_drain_and_barrier

    f32 = mybir.dt.float32
    f32r = mybir.dt.float32r
    AF = mybir.ActivationFunctionType
    OP = mybir.AluOpType

    MM_F32R = False  # walrus requires f32r inputs rounded at the producer

    def r(ap):
        return ap.bitcast(f32r) if MM_F32R else ap

    nc = bass.Bass()

    def dram_in(name, shape):
        return nc.dram_tensor(name, list(shape), f32, kind="ExternalInput")

    in1 = dram_in("in1", (G, D))
    in2 = dram_in("in2", (G, D))
    Md = dram_in("M", (G, G))
    ln1_g = dram_in("ln1_g", (D,)); ln1_b = dram_in("ln1_b", (D,))
    ln2_g = dram_in("ln2_g", (D,)); ln2_b = dram_in("ln2_b", (D,))
    wq1 = dram_in("wq1", (D, D)); bq1 = dram_in("bq1", (D,))
    wk1 = dram_in("wk1", (D, D)); bk1 = dram_in("bk1", (D,))
    wq2 = dram_in("wq2", (D, D)); bq2 = dram_in("bq2", (D,))
    wk2 = dram_in("wk2", (D, D)); bk2 = dram_in("bk2", (D,))
    wv2 = dram_in("wv2", (D, D)); bv2 = dram_in("bv2", (D,))
    fwq = dram_in("fwq", (2 * HD, HD)); fbq = dram_in("fbq", (HD,))
    fwk = dram_in("fwk", (2 * HD, HD)); fbk = dram_in("fbk", (HD,))
    wo = dram_in("wo", (D, D)); bo = dram_in("bo", (D,))
    w1 = dram_in("w1", (D, DFF)); b1 = dram_in("b1", (DFF,))
    w2 = dram_in("w2", (DFF, D)); b2 = dram_in("b2", (D,))
    coef = dram_in("coef", (2,))

    out_d = nc.dram_tensor("out", [G, D], f32, kind="ExternalOutput")

    with tile.TileContext(nc) as tc:
        with contextlib.ExitStack() as ctx:
            pc = ctx.enter_context(tc.tile_pool(name="const", bufs=1))
            ident = pc.tile([P, P], f32, tag="ident")
            make_identity(nc, ident)
            eps_t = pc.tile([P, 1], f32, tag="eps")
            nc.vector.memset(eps_t, EPS)
            cA_t = pc.tile([P, 1], f32, tag="cA")
            nc.sync.dma_start(out=cA_t, in_=coef[None, 0:1].to_broadcast([P, 1]))
            cB_t = pc.tile([P, 1], f32, tag="cB")
            nc.sync.dma_start(out=cB_t, in_=coef[None, 1:2].to_broadcast([P, 1]))

            def rep_row(name, vec, n=D):  # [n] dram -> [P, n] replicated rows
                t = pc.tile([P, n], f32, tag=name)
                nc.sync.dma_start(out=t, in_=vec[None, :].to_broadcast([P, n]))
                return t

            g1_rep = rep_row("g1_rep", ln1_g); b1_rep = rep_row("b1_rep", ln1_b)
            g2_rep = rep_row("g2_rep", ln2_g); b2_rep = rep_row("b2_rep", ln2_b)
            bo_rep = rep_row("bo_rep", bo)
            bv2_rep = rep_row("bv2_rep", bv2)

            def col(name, vec, n):  # [n*P] dram -> [P, n] column tile
                t = pc.tile([P, n], f32, tag=name)
                nc.sync.dma_start(out=t, in_=vec.rearrange("(c p) -> p c", p=P))
                return t

            bq1_c = col("bq1_c", bq1, KC); bk1_c = col("bk1_c", bk1, KC)
            bq2_c = col("bq2_c", bq2, KC); bk2_c = col("bk2_c", bk2, KC)
            b1_c = col("b1_c", b1, FC); b2_c = col("b2_c", b2, KC)
            fbq_c = pc.tile([HD, 1], f32, tag="fbq_c")
            nc.sync.dma_start(out=fbq_c, in_=fbq[:, None])
            fbk_c = pc.tile([HD, 1], f32, tag="fbk_c")
            nc.sync.dma_start(out=fbk_c, in_=fbk[:, None])
            ones_row = pc.tile([1, HD], f32, tag="ones_row")
            nc.vector.memset(ones_row, 1.0)

            # long-lived attention operands (packed head layout: head h ->
            # partition rows (h%2)*64, chunk h//2)
            # survives into phase 3
            pD = ctx.enter_context(tc.tile_pool(name="resid", bufs=1))
            OT = pD.tile([P, KC, G], f32, tag="OT")         # merged heads, feature-major
            h_res = pD.tile([P, ICH, D], f32, tag="h_res")  # token-major residual
            ST_d1 = pD.tile([H, G], f32, tag="ST_d1")       # row h: d1_h
            ST_s2 = pD.tile([H, G], f32, tag="ST_s2")       # row h: s2_h
            scale_rows = pD.tile([H, G], f32, tag="scale_rows")

            # long-lived attention operands (packed head layout: head h ->
            # partition rows (h%2)*64, chunk h//2); freed before phase 3
            pB_cm = tc.tile_pool(name="attn_ops", bufs=1)
            pB = pB_cm.__enter__()
            QfT = pB.tile([P, H // 2, G], f32, tag="QfT")
            KfT = pB.tile([P, H // 2, G], f32, tag="KfT")
            V_st = pB.tile([P, ICH, H, HD + 1], f32, tag="V_st")  # [j, jc, h, 65]

            ones_col = V_st[:, 0, 0, HD:HD + 1]   # [128, 1] of ones
            nc.vector.memset(V_st, 1.0)           # ones cols; V parts overwritten

            def ln_chunk(src_ap, xg_out, wkp, g_rep_t, b_rep_t):
                """LayerNorm of a [P, D] token-major chunk into xg_out."""
                stats = wkp.tile([P, 6], f32, tag="ln_stats")
                mv = wkp.tile([P, 2], f32, tag="ln_mv")
                nc.vector.bn_stats(out=stats, in_=src_ap)
                nc.vector.bn_aggr(out=mv, in_=stats)
                stdt = wkp.tile([P, 1], f32, tag="ln_std")
                nc.scalar.activation(out=stdt, in_=mv[:, 1:2], func=AF.Sqrt,
                                     bias=eps_t, scale=1.0)
                rstd = wkp.tile([P, 1], f32, tag="ln_rstd")
                nc.vector.reciprocal(out=rstd, in_=stdt)
                xn = wkp.tile([P, D], f32, tag="ln_xn")
                nc.vector.tensor_scalar(out=xn, in0=src_ap, scalar1=mv[:, 0:1],
                                        scalar2=rstd, op0=OP.subtract, op1=OP.mult)
                nc.gpsimd.tensor_tensor(out=xn, in0=xn, in1=g_rep_t, op=OP.mult)
                nc.gpsimd.tensor_tensor(out=xg_out, in0=xn, in1=b_rep_t, op=OP.add)

            # ============ phase 1: LN1, projections, fused Q/K ============
            with tc.tile_pool(name="p1", bufs=1) as p1, \
                 tc.tile_pool(name="p1w", bufs=2) as p1w, \
                 tc.tile_pool(name="p1ps", bufs=4, space="PSUM") as p1ps:

                xgeT = p1.tile([P, KC, G], f32, tag="xgeT")
                xeeT = p1.tile([P, KC, G], f32, tag="xeeT")
                for (src, dstT) in ((in1, xgeT), (in2, xeeT)):
                    for ic in range(ICH):
                        xc = p1w.tile([P, D], f32, tag="ln_in")
                        nc.sync.dma_start(out=xc, in_=src[ic * P:(ic + 1) * P, :])
                        xg = p1w.tile([P, D], f32, tag="ln_out")
                        ln_chunk(xc, xg, p1w, g1_rep, b1_rep)
                        pt = p1ps.tile([P, 512], f32, tag="ps")
                        for kc in range(KC):
                            nc.tensor.transpose(pt[:, kc * P:(kc + 1) * P],
                                                xg[:, kc * P:(kc + 1) * P], ident)
                        nc.scalar.activation(
                            out=dstT[:, :, ic * P:(ic + 1) * P],
                            in_=pt.rearrange("p (c i) -> p c i", i=P),
                            func=AF.Copy)

                def load_w(name, wd):
                    t = p1.tile([P, KC, D], f32, tag=name)
                    nc.sync.dma_start(out=t, in_=wd.rearrange("(c p) n -> p c n", p=P))
                    return t

                wq1_t = load_w("wq1_t", wq1)
                wk1_t = load_w("wk1_t", wk1)
                wq2_t = load_w("wq2_t", wq2)
                wk2_t = load_w("wk2_t", wk2)
                wv2_t = load_w("wv2_t", wv2)
                fwq_t = p1.tile([P, HD], f32, tag="fwq_t")
                nc.sync.dma_start(out=fwq_t, in_=fwq[:, :])
                fwk_t = p1.tile([P, HD], f32, tag="fwk_t")
                nc.sync.dma_start(out=fwk_t, in_=fwk[:, :])

                # V projection (token-major) into V_st slots
                for jc in range(ICH):
                    ps = p1ps.tile([P, D], f32, tag="ps")
                    for kc in range(KC):
                        nc.tensor.matmul(ps,
                                         r(xeeT[:, kc, jc * P:(jc + 1) * P]),
                                         r(wv2_t[:, kc, :]),
                                         start=(kc == 0), stop=(kc == KC - 1))
                    nc.vector.tensor_tensor(
                        out=V_st[:, jc, :, 0:HD],
                        in0=ps.rearrange("p (h d) -> p h d", d=HD),
                        in1=bv2_rep.rearrange("p (h d) -> p h d", d=HD),
                        op=OP.add)

                # Q/K projections + fused projection, per head-pair chunk c
                for c in range(KC):
                    Qst = {}; Kst = {}
                    for h in (2 * c, 2 * c + 1):
                        Qst[h] = p1w.tile([P, G], f32, tag="Qst", name=f"Qst_{h}")
                        Kst[h] = p1w.tile([P, G], f32, tag="Kst", name=f"Kst_{h}")
                    for (wt, bias_c, st, half) in (
                        (wq1_t, bq1_c, Qst, 0), (wq2_t, bq2_c, Qst, 1),
                        (wk1_t, bk1_c, Kst, 0), (wk2_t, bk2_c, Kst, 1),
                    ):
                        srcT = xgeT if half == 0 else xeeT
                        for ih in range(NIH):
                            ps = p1ps.tile([P, IH], f32, tag="ps")
                            for kc in range(KC):
                                nc.tensor.matmul(
                                    ps,
                                    r(wt[:, kc, c * P:(c + 1) * P]),
                                    r(srcT[:, kc, ih * IH:(ih + 1) * IH]),
                                    start=(kc == 0), stop=(kc == KC - 1))
                            # psum rows 0:64 -> head 2c, 64:128 -> head 2c+1;
                            # stacked tiles: g-side rows 0:64, e-side 64:128
                            for hh in range(2):
                                h = 2 * c + hh
                                nc.scalar.activation(
                                    out=st[h][half * HD:(half + 1) * HD,
                                              ih * IH:(ih + 1) * IH],
                                    in_=ps[hh * HD:(hh + 1) * HD, :],
                                    func=AF.Identity,
                                    bias=bias_c[hh * HD:(hh + 1) * HD, c:c + 1],
                                    scale=1.0)
                    for h in (2 * c, 2 * c + 1):
                        hr = (h % 2) * HD
                        hc = h // 2
                        for (st, ft, fb, dstT) in ((Qst, fwq_t, fbq_c, QfT),
                                                   (Kst, fwk_t, fbk_c, KfT)):
                            for ih in range(NIH):
                                ps = p1ps.tile([HD, IH], f32, tag="psf")
                                nc.tensor.matmul(ps, r(ft),
                                                 r(st[h][:, ih * IH:(ih + 1) * IH]),
                                                 start=True, stop=True)
                                nc.scalar.activation(
                                    out=dstT[hr:hr + HD, hc, ih * IH:(ih + 1) * IH],
                                    in_=ps, func=AF.Identity, bias=fb, scale=1.0)

            # ================= phase 2: attention =========================
            with tc.tile_pool(name="pMT", bufs=1) as pMT, \
                 tc.tile_pool(name="p2w", bufs=2) as p2w, \
                 tc.tile_pool(name="p2s", bufs=2, space="PSUM") as p2s, \
                 tc.tile_pool(name="p2o", bufs=2, space="PSUM") as p2o, \
                 tc.tile_pool(name="p2d", bufs=2, space="PSUM") as p2d:

                MT = pMT.tile([P, ICH, G], f32, tag="MT")   # [j-part, jc, i]
                for ic in range(ICH):
                    mrow = p2w.tile([P, G], f32, tag="mrow")
                    nc.sync.dma_start(out=mrow, in_=Md[ic * P:(ic + 1) * P, :])
                    for jh in range(2):
                        pt = p2s.tile([P, 512], f32, tag="sp")
                        for k in range(4):
                            jc = jh * 4 + k
                            nc.tensor.transpose(pt[:, k * P:(k + 1) * P],
                                                mrow[:, jc * P:(jc + 1) * P], ident)
                        nc.scalar.activation(
                            out=MT[:, jh * 4:(jh + 1) * 4, ic * P:(ic + 1) * P],
                            in_=pt.rearrange("p (c i) -> p c i", i=P),
                            func=AF.Copy)

                for h in range(H):
                    hr = (h % 2) * HD
                    hc = h // 2
                    oa = [p2o.tile([HD + 1, IH], f32, tag="oa", name=f"oa_{h}_{i}") for i in range(NIH)]
                    d1 = [p2d.tile([1, IH], f32, tag="d1", name=f"d1_{h}_{i}") for i in range(NIH)]
                    for jc in range(ICH):
                        et = p2w.tile([P, G], f32, tag="et")
                        pt_ = p2w.tile([P, G], f32, tag="pt")
                        for ih in range(NIH):
                            sp = p2s.tile([P, IH], f32, tag="sp")
                            nc.tensor.matmul(
                                sp,
                                r(KfT[hr:hr + HD, hc, jc * P:(jc + 1) * P]),
                                r(QfT[hr:hr + HD, hc, ih * IH:(ih + 1) * IH]),
                                start=True, stop=True)
                            nc.scalar.activation(out=et[:, ih * IH:(ih + 1) * IH],
                                                 in_=sp, func=AF.Exp, scale=SCALE)
                        nc.vector.tensor_tensor(out=pt_, in0=et, in1=MT[:, jc, :],
                                                op=OP.mult)
                        for ih in range(NIH):
                            nc.tensor.matmul(oa[ih],
                                             r(V_st[:, jc, h, :]),
                                             r(pt_[:, ih * IH:(ih + 1) * IH]),
                                             start=(jc == 0), stop=(jc == ICH - 1))
                            nc.tensor.matmul(d1[ih],
                                             r(ones_col),
                                             r(et[:, ih * IH:(ih + 1) * IH]),
                                             start=(jc == 0), stop=(jc == ICH - 1))
                    for ih in range(NIH):
                        isl = slice(ih * IH, (ih + 1) * IH)
                        nc.scalar.activation(out=OT[hr:hr + HD, hc, isl],
                                             in_=oa[ih][0:HD, :], func=AF.Copy)
                        # stage single stat rows at base partition 0, then DMA
                        # into the stacked stat tiles (engines need 32-aligned
                        # partition bases; DMA does not)
                        s2st = p2w.tile([1, IH], f32, tag="s2st",
                                        name=f"s2st_{h}_{ih}")
                        nc.scalar.activation(out=s2st, in_=oa[ih][HD:HD + 1, :],
                                             func=AF.Copy)
                        nc.sync.dma_start(out=ST_s2[h:h + 1, isl], in_=s2st)
                        d1st = p2w.tile([1, IH], f32, tag="d1st",
                                        name=f"d1st_{h}_{ih}")
                        nc.scalar.activation(out=d1st, in_=d1[ih], func=AF.Copy)
                        nc.sync.dma_start(out=ST_d1[h:h + 1, isl], in_=d1st)

                # scale_rows = 1 / (cA*s2 + cB*d1)
                t_a = p2w.tile([H, G], f32, tag="t_a")
                nc.vector.tensor_scalar_mul(out=t_a, in0=ST_s2, scalar1=cA_t[0:H])
                t_b = p2w.tile([H, G], f32, tag="t_b")
                nc.vector.tensor_scalar_mul(out=t_b, in0=ST_d1, scalar1=cB_t[0:H])
                nc.vector.tensor_add(out=t_a, in0=t_a, in1=t_b)
                nc.vector.reciprocal(out=scale_rows, in_=t_a)

                # apply per-(head, i) scale to OT rows: replicate the scale
                # row across 64 partitions with a k=1 ones-matmul into PSUM
                for h in range(H):
                    hr = (h % 2) * HD
                    hc = h // 2
                    srow = p2w.tile([1, G], f32, tag="srow", name=f"srow_{h}")
                    nc.sync.dma_start(out=srow, in_=scale_rows[h:h + 1, :])
                    for ih in range(NIH):
                        isl = slice(ih * IH, (ih + 1) * IH)
                        srep = p2s.tile([HD, IH], f32, tag="sp",
                                        name=f"srep_{h}_{ih}")
                        nc.tensor.matmul(srep, r(ones_row), r(srow[:, isl]),
                                         start=True, stop=True)
                        nc.vector.tensor_tensor(out=OT[hr:hr + HD, hc, isl],
                                                in0=OT[hr:hr + HD, hc, isl],
                                                in1=srep, op=OP.mult)

            # ============ phase 3: out-proj, residual, LN2, FFN ===========
            pB_cm.__exit__(None, None, None)
            with tc.tile_pool(name="p3", bufs=1) as p3, \
                 tc.tile_pool(name="p3w", bufs=3) as p3w, \
                 tc.tile_pool(name="p3s", bufs=2) as p3s, \
                 tc.tile_pool(name="p3ps", bufs=4, space="PSUM") as p3ps:

                wo_t = p3.tile([P, KC, D], f32, tag="wo_t")
                nc.sync.dma_start(out=wo_t, in_=wo.rearrange("(c p) n -> p c n", p=P))
                x2T = p3.tile([P, KC, G], f32, tag="x2T")

                for ic in range(ICH):
                    ps = p3ps.tile([P, D], f32, tag="ps")
                    for dc in range(KC):
                        nc.tensor.matmul(ps,
                                         r(OT[:, dc, ic * P:(ic + 1) * P]),
                                         r(wo_t[:, dc, :]),
                                         start=(dc == 0), stop=(dc == KC - 1))
                    in2c = p3w.tile([P, D], f32, tag="in2c")
                    nc.sync.dma_start(out=in2c, in_=in2[ic * P:(ic + 1) * P, :])
                    t0 = p3w.tile([P, D], f32, tag="res_t0")
                    nc.vector.tensor_add(out=t0, in0=ps, in1=in2c)
                    nc.gpsimd.tensor_tensor(out=h_res[:, ic, :], in0=t0,
                                            in1=bo_rep, op=OP.add)
                    x2c = p3w.tile([P, D], f32, tag="x2c")
                    ln_chunk(h_res[:, ic, :], x2c, p3w, g2_rep, b2_rep)
                    pt = p3ps.tile([P, 512], f32, tag="ps")
                    for kc in range(KC):
                        nc.tensor.transpose(pt[:, kc * P:(kc + 1) * P],
                                            x2c[:, kc * P:(kc + 1) * P], ident)
                    nc.scalar.activation(
                        out=x2T[:, :, ic * P:(ic + 1) * P],
                        in_=pt.rearrange("p (c i) -> p c i", i=P),
                        func=AF.Copy)

                h1g = p3.tile([P, FC, IH], f32, tag="h1g")
                o2T = p3.tile([P, KC, IH], f32, tag="o2T")
                for ih in range(NIH):
                    isl = slice(ih * IH, (ih + 1) * IH)
                    for fc in range(FC):
                        w1_fc = p3s.tile([P, KC, P], f32, tag="w1_fc")
                        nc.sync.dma_start(
                            out=w1_fc,
                            in_=w1[:, fc * P:(fc + 1) * P]
                                .rearrange("(c p) n -> p c n", p=P))
                        ps = p3ps.tile([P, IH], f32, tag="ps")
                        for kc in range(KC):
                            nc.tensor.matmul(ps,
                                             r(w1_fc[:, kc, :]),
                                             r(x2T[:, kc, isl]),
                                             start=(kc == 0), stop=(kc == KC - 1))
                        nc.scalar.activation(out=h1g[:, fc, :], in_=ps, func=AF.Gelu,
                                             bias=b1_c[:, fc:fc + 1], scale=1.0)
                    for dc in range(KC):
                        w2_dc = p3s.tile([P, FC, P], f32, tag="w2_dc")
                        nc.sync.dma_start(
                            out=w2_dc,
                            in_=w2[:, dc * P:(dc + 1) * P]
                                .rearrange("(c p) n -> p c n", p=P))
                        ps = p3ps.tile([P, IH], f32, tag="ps")
                        for fc in range(FC):
                            nc.tensor.matmul(ps,
                                             r(w2_dc[:, fc, :]),
                                             r(h1g[:, fc, :]),
                                             start=(fc == 0), stop=(fc == FC - 1))
                        nc.scalar.activation(out=o2T[:, dc, :], in_=ps,
                                             func=AF.Identity,
                                             bias=b2_c[:, dc:dc + 1], scale=1.0)
                    for icl in range(KC):  # 4 token chunks in this half
                        ic = ih * KC + icl
                        pt = p3ps.tile([P, 512], f32, tag="ps")
                        for dc in range(KC):
                            nc.tensor.transpose(pt[:, dc * P:(dc + 1) * P],
                                                o2T[:, dc, icl * P:(icl + 1) * P],
                                                ident)
                        outc = p3w.tile([P, D], f32, tag="outc")
                        nc.vector.tensor_add(out=outc, in0=pt, in1=h_res[:, ic, :])
                        nc.sync.dma_start(out=out_d[ic * P:(ic + 1) * P, :], in_=outc)

    if split_waits:
        _split_sync_waits(nc, mybir)
    return nc


def _split_sync_waits(nc, mybir, maxw=1):
    """walrus CoreV3 codegen allows only one sem wait per instruction; move
    excess waits onto same-engine nops inserted before the instruction."""
    nid = 0
    for fn in nc.m.functions:
        for blk in fn.blocks:
            orig = list(blk.instructions)
            if not any(i.sync_info and i.sync_info.on_wait and
                       len(i.sync_info.on_wait) > maxw for i in orig):
                continue
            new = []
            for ins in orig:
                si = ins.sync_info
                waits = list(si.on_wait) if si and si.on_wait else []
                if len(waits) > maxw:
                    si.on_wait = waits[:maxw]
                    for k in range(maxw, len(waits), maxw):
                        nop = mybir.InstNoOp(name=f"I-wsplit-{nid}", ins=[], outs=[])
                        nid += 1
                        nop.engine = ins.engine
                        nop.sync_info = mybir.SyncInfo(
                            on_wait=waits[k:k + maxw], on_update=[])
                        new.append(nop)
                new.append(ins)
            blk.instructions = new


def _in_maps(inputs):
    inp = {k: np.ascontiguousarray(np.asarray(v, np.float32)) for k, v in inputs.items()}
    I64 = np.eye(HD, dtype=np.float32)
    sel2 = np.concatenate([np.zeros((HD, HD), np.float32), I64], 0)
    z64 = np.zeros(HD, np.float32)

    maps = []
    for core in range(8):
        b = core % 4
        if core < 4:  # gene branch
            m = dict(
                in1=inp["gene_emb"][b], in2=inp["gene_emb"][b], M=inp["M"][b],
                ln1_g=inp["ln_g1_g"], ln1_b=inp["ln_g1_b"],
                ln2_g=inp["ln_g2_g"], ln2_b=inp["ln_g2_b"],
                wq1=inp["gene_wq"], bq1=inp["gene_bq"],
                wk1=inp["gene_wk"], bk1=inp["gene_bk"],
                wq2=inp["gene_wq"], bq2=inp["gene_bq"],
                wk2=inp["gene_wk"], bk2=inp["gene_bk"],
                wv2=inp["gene_wv"], bv2=inp["gene_bv"],
                fwq=sel2, fbq=z64, fwk=sel2, fbk=z64,
                wo=inp["gout_w"], bo=inp["gout_b"],
                w1=inp["ffn_g_w1"], b1=inp["ffn_g_b1"],
                w2=inp["ffn_g_w2"], b2=inp["ffn_g_b2"],
                coef=np.array([1.0, 1e-8], np.float32),
            )
        else:  # expr branch
            m = dict(
                in1=inp["gene_emb"][b], in2=inp["expr_emb"][b], M=inp["M"][b],
                ln1_g=inp["ln_e1_g"], ln1_b=inp["ln_e1_b"],
                ln2_g=inp["ln_e2_g"], ln2_b=inp["ln_e2_b"],
                wq1=inp["gene_wq"], bq1=inp["gene_bq"],
                wk1=inp["gene_wk"], bk1=inp["gene_bk"],
                wq2=inp["expr_wq"], bq2=inp["expr_bq"],
                wk2=inp["expr_wk"], bk2=inp["expr_bk"],
                wv2=inp["expr_wv"], bv2=inp["expr_bv"],
                fwq=inp["fused_wq"], fbq=inp["fused_bq"],
                fwk=inp["fused_wk"], fbk=inp["fused_bk"],
                wo=inp["eout_w"], bo=inp["eout_b"],
                w1=inp["ffn_e_w1"], b1=inp["ffn_e_b1"],
                w2=inp["ffn_e_w2"], b2=inp["ffn_e_b2"],
                coef=np.array([0.0, 1.0], np.float32),
            )
        maps.append({k: np.ascontiguousarray(v) for k, v in m.items()})
    return maps


def kernel(**inputs):
    from concourse.bass_utils import run_bass_kernel_spmd

    if "nc" not in _cache:
        _cache["nc"] = _build_program()
    nc = _cache["nc"]

    res = run_bass_kernel_spmd(nc, _in_maps(inputs), core_ids=list(range(8)))
    out_gene = np.stack([res.results[c]["out"] for c in range(4)])
    out_expr = np.stack([res.results[c]["out"] for c in range(4, 8)])
    return (out_gene, out_expr)



# revision 3
# speedup vs baseline: 2.2964x; 2.2964x over previous
"""Trainium2 Bass kernel for nn_DeepSCTransformerBlock.

Sharding: 8 cores = 4 batches x 2 branches (gene/expr). Zero collectives.
One SPMD program; the gene branch is expressed through the expr skeleton by
host-side weight folding, and the (s2, d1) normalization coefficients
(cA, cB) are per-core input data:
  gene: scale_i = 1/(1*s2 + 1e-8*d1)   [signed-L1 renorm of softmax*mask]
  expr: scale_i = 1/(0*s2 + 1*d1)      [plain softmax denominator]
where E = exp(S), P = E*M, d1 = colsum(E), s2 = colsum(P); O = scale * (P @ V).

The fused head projection Qf = concat(Q1, Q2) @ fwq + fbq is folded on the
host into two D x D weights (per-head block product), so Qf/Kf come out of a
single accumulated projection: QfT = Wcq1^T @ x1T + Wcq2^T @ x2T + b.

All matmul operands are bf16 (1 PE cycle/row vs 4 for fp32); PSUM accumulates
fp32; the residual stream, LayerNorm statistics, and softmax scale math stay
fp32. Activations flow feature-major ([d, tokens]) through matmul chains; S is
computed transposed ([j, i]) so the AV contraction needs no transposes of the
attention matrix; s2 falls out of a ones-column appended to V.
"""

import numpy as np

B, G, D, H = 4, 1024, 512, 8
HD = D // H
DFF = 4 * D
P = 128
ICH = G // P      # 8 token chunks
KC = D // P       # 4 feature chunks
FC = DFF // P     # 16 ffn-hidden chunks
NIH = 2           # i halves (free-dim 512 per matmul)
IH = G // NIH     # 512
SCALE = 1.0 / (HD ** 0.5)
EPS = 1e-5

_cache = {}


def _build_program(split_waits=True):
    import contextlib
    import concourse.bass as bass
    import concourse.mybir as mybir
    import concourse.tile as tile
    from concourse.masks import make_identity

    # walrus CoreV3 codegen rejects instructions carrying >1 sem wait at the
    # Tile end-of-kernel drain; split the waits across single-wait nops.
    def _patched_drain_and_barrier(self, tick_clock, wait_clock):
        nc = self.nc
        drain_inst = nc.sync.drain()
        wait_clock.add_sem_waits(
            drain_inst.ins, tile.ScopedClock({None: tick_clock.global_clock})
        )
        si = drain_inst.ins.sync_info
        if si is not None and si.on_wait and len(si.on_wait) > 1:
            waits = list(si.on_wait)
            si.on_wait = waits[:1]
            for i in range(1, len(waits)):
                nop = nc.sync.nop(hint="drain_wait_split", nofuse=True)
                nop.ins.sync_info = mybir.SyncInfo(
                    on_wait=waits[i : i + 1], on_update=[]
                )
        nc.all_engine_barrier()
        assert self.sems is not None
        popped = nc._tile_sem_poison_stack.pop()
        assert popped is self._sem_poison
        nc.clear_and_free_semaphores(list(self.sems.allocated().values()))
        nc.all_engine_barrier()

    tile.TileContext._drain_and_barrier = _patched_drain_and_barrier

    f32 = mybir.dt.float32
    bf16 = mybir.dt.bfloat16
    AF = mybir.ActivationFunctionType
    OP = mybir.AluOpType

    nc = bass.Bass()

    def dram_in(name, shape, dt=f32):
        return nc.dram_tensor(name, list(shape), dt, kind="ExternalInput")

    in1 = dram_in("in1", (G, D))
    in2 = dram_in("in2", (G, D))
    Md = dram_in("M", (G, G), bf16)
    ln1_g = dram_in("ln1_g", (D,)); ln1_b = dram_in("ln1_b", (D,))
    ln2_g = dram_in("ln2_g", (D,)); ln2_b = dram_in("ln2_b", (D,))
    wcq1 = dram_in("wcq1", (D, D), bf16)
    wcq2 = dram_in("wcq2", (D, D), bf16)
    wck1 = dram_in("wck1", (D, D), bf16)
    wck2 = dram_in("wck2", (D, D), bf16)
    fbq = dram_in("fbq", (D,))
    fbk = dram_in("fbk", (D,))
    wv2 = dram_in("wv2", (D, D), bf16); bv2 = dram_in("bv2", (D,))
    wo = dram_in("wo", (D, D), bf16); bo = dram_in("bo", (D,))
    w1 = dram_in("w1", (D, DFF), bf16); b1 = dram_in("b1", (DFF,))
    w2 = dram_in("w2", (DFF, D), bf16); b2 = dram_in("b2", (D,))
    coef = dram_in("coef", (2,))

    out_d = nc.dram_tensor("out", [G, D], f32, kind="ExternalOutput")

    with tile.TileContext(nc) as tc:
        with contextlib.ExitStack() as ctx:
            pc = ctx.enter_context(tc.tile_pool(name="const", bufs=1))
            identb = pc.tile([P, P], bf16, tag="identb")
            make_identity(nc, identb)
            eps_t = pc.tile([P, 1], f32, tag="eps")
            nc.vector.memset(eps_t, EPS)
            cA_t = pc.tile([P, 1], f32, tag="cA")
            nc.sync.dma_start(out=cA_t, in_=coef[None, 0:1].to_broadcast([P, 1]))
            cB_t = pc.tile([P, 1], f32, tag="cB")
            nc.sync.dma_start(out=cB_t, in_=coef[None, 1:2].to_broadcast([P, 1]))

            def rep_row(name, vec, n=D):  # [n] dram -> [P, n] replicated rows
                t = pc.tile([P, n], f32, tag=name)
                nc.sync.dma_start(out=t, in_=vec[None, :].to_broadcast([P, n]))
                return t

            g1_rep = rep_row("g1_rep", ln1_g); b1_rep = rep_row("b1_rep", ln1_b)
            g2_rep = rep_row("g2_rep", ln2_g); b2_rep = rep_row("b2_rep", ln2_b)
            bo_rep = rep_row("bo_rep", bo)
            bv2_rep = rep_row("bv2_rep", bv2)

            def col(name, vec, n):  # [n*P] dram -> [P, n] column tile
                t = pc.tile([P, n], f32, tag=name)
                nc.sync.dma_start(out=t, in_=vec.rearrange("(c p) -> p c", p=P))
                return t

            fbq_c = col("fbq_c", fbq, KC); fbk_c = col("fbk_c", fbk, KC)
            b1_c = col("b1_c", b1, FC); b2_c = col("b2_c", b2, KC)
            ones_row = pc.tile([1, HD], bf16, tag="ones_row")
            nc.vector.memset(ones_row, 1.0)

            # residual stream and attention stats (token-major), live to end
            pD = ctx.enter_context(tc.tile_pool(name="resid", bufs=1))
            OT = pD.tile([P, KC, G], bf16, tag="OT")        # merged heads, feature-major
            h_res = pD.tile([P, ICH, D], f32, tag="h_res")  # token-major residual
            ST_d1 = pD.tile([H, G], f32, tag="ST_d1")       # row h: d1_h
            ST_s2 = pD.tile([H, G], f32, tag="ST_s2")       # row h: s2_h
            scale_rows = pD.tile([H, G], f32, tag="scale_rows")
            scale_bf = pD.tile([H, G], bf16, tag="scale_bf")

            # long-lived attention operands (packed head layout: head h ->
            # partition rows (h%2)*64, chunk h//2); freed before phase 3
            pB_cm = tc.tile_pool(name="attn_ops", bufs=1)
            pB = pB_cm.__enter__()
            QfT = pB.tile([P, H // 2, G], bf16, tag="QfT")
            KfT = pB.tile([P, H // 2, G], bf16, tag="KfT")
            V_st = pB.tile([P, ICH, H, HD + 1], bf16, tag="V_st")  # [j, jc, h, 65]

            ones_col = V_st[:, 0, 0, HD:HD + 1]   # [128, 1] of ones
            nc.vector.memset(V_st, 1.0)           # ones cols; V parts overwritten

            def ln_chunk(src_ap, xg_out, wkp, g_rep_t, b_rep_t):
                """LayerNorm of a [P, D] token-major fp32 chunk into bf16."""
                stats = wkp.tile([P, 6], f32, tag="ln_stats")
                mv = wkp.tile([P, 2], f32, tag="ln_mv")
                nc.vector.bn_stats(out=stats, in_=src_ap)
                nc.vector.bn_aggr(out=mv, in_=stats)
                stdt = wkp.tile([P, 1], f32, tag="ln_std")
                nc.scalar.activation(out=stdt, in_=mv[:, 1:2], func=AF.Sqrt,
                                     bias=eps_t, scale=1.0)
                rstd = wkp.tile([P, 1], f32, tag="ln_rstd")
                nc.vector.reciprocal(out=rstd, in_=stdt)
                xn = wkp.tile([P, D], f32, tag="ln_xn")
                nc.vector.tensor_scalar(out=xn, in0=src_ap, scalar1=mv[:, 0:1],
                                        scalar2=rstd, op0=OP.subtract, op1=OP.mult)
                nc.gpsimd.tensor_tensor(out=xn, in0=xn, in1=g_rep_t, op=OP.mult)
                nc.gpsimd.tensor_tensor(out=xg_out, in0=xn, in1=b_rep_t, op=OP.add)

            # ============ phase 1: LN1, V proj, folded Qf/Kf proj ==========
            with tc.tile_pool(name="p1", bufs=1) as p1, \
                 tc.tile_pool(name="p1w", bufs=2) as p1w, \
                 tc.tile_pool(name="p1ps", bufs=4, space="PSUM") as p1ps, \
                 tc.tile_pool(name="p1pt", bufs=2, space="PSUM") as p1pt:

                xgeT = p1.tile([P, KC, G], bf16, tag="xgeT")
                xeeT = p1.tile([P, KC, G], bf16, tag="xeeT")
                for (src, dstT) in ((in1, xgeT), (in2, xeeT)):
                    for ic in range(ICH):
                        xc = p1w.tile([P, D], f32, tag="ln_in")
                        nc.sync.dma_start(out=xc, in_=src[ic * P:(ic + 1) * P, :])
                        xg = p1w.tile([P, D], bf16, tag="ln_out")
                        ln_chunk(xc, xg, p1w, g1_rep, b1_rep)
                        pt = p1pt.tile([P, D], bf16, tag="pt")
                        for kc in range(KC):
                            nc.tensor.transpose(pt[:, kc * P:(kc + 1) * P],
                                                xg[:, kc * P:(kc + 1) * P], identb)
                        nc.scalar.activation(
                            out=dstT[:, :, ic * P:(ic + 1) * P],
                            in_=pt.rearrange("p (c i) -> p c i", i=P),
                            func=AF.Copy)

                def load_w(name, wd):
                    t = p1.tile([P, KC, D], bf16, tag=name)
                    nc.sync.dma_start(out=t, in_=wd.rearrange("(c p) n -> p c n", p=P))
                    return t

                wcq1_t = load_w("wcq1_t", wcq1)
                wcq2_t = load_w("wcq2_t", wcq2)
                wck1_t = load_w("wck1_t", wck1)
                wck2_t = load_w("wck2_t", wck2)
                wv2_t = load_w("wv2_t", wv2)

                # V projection (token-major) into V_st slots
                for jc in range(ICH):
                    ps = p1ps.tile([P, D], f32, tag="ps")
                    for kc in range(KC):
                        nc.tensor.matmul(ps,
                                         xeeT[:, kc, jc * P:(jc + 1) * P],
                                         wv2_t[:, kc, :],
                                         start=(kc == 0), stop=(kc == KC - 1))
                    nc.vector.tensor_tensor(
                        out=V_st[:, jc, :, 0:HD],
                        in0=ps.rearrange("p (h d) -> p h d", d=HD),
                        in1=bv2_rep.rearrange("p (h d) -> p h d", d=HD),
                        op=OP.add)

                # folded Qf/Kf projections, per head-pair chunk c:
                # QfT[:,c] = Wcq1[:,c]^T @ xgeT + Wcq2[:,c]^T @ xeeT + fbq
                for c in range(KC):
                    csl = slice(c * P, (c + 1) * P)
                    for (wt1, wt2, bias_c, dstT) in (
                        (wcq1_t, wcq2_t, fbq_c, QfT),
                        (wck1_t, wck2_t, fbk_c, KfT),
                    ):
                        pss = [p1ps.tile([P, IH], f32, tag="ps",
                                         name=f"psqk_{c}_{ih}") for ih in range(NIH)]
                        for si, (wt, srcT) in enumerate(((wt1, xgeT), (wt2, xeeT))):
                            for kc in range(KC):
                                st = (si == 0 and kc == 0)
                                sp_ = (si == 1 and kc == KC - 1)
                                for ih in range(NIH):
                                    nc.tensor.matmul(
                                        pss[ih],
                                        wt[:, kc, csl],
                                        srcT[:, kc, ih * IH:(ih + 1) * IH],
                                        start=st, stop=sp_)
                        for ih in range(NIH):
                            nc.scalar.activation(
                                out=dstT[:, c, ih * IH:(ih + 1) * IH],
                                in_=pss[ih], func=AF.Identity,
                                bias=bias_c[:, c:c + 1], scale=1.0)

            # ================= phase 2: attention =========================
            with tc.tile_pool(name="pMT", bufs=1) as pMT, \
                 tc.tile_pool(name="p2w", bufs=2) as p2w, \
                 tc.tile_pool(name="p2s", bufs=2, space="PSUM") as p2s, \
                 tc.tile_pool(name="p2o", bufs=2, space="PSUM") as p2o, \
                 tc.tile_pool(name="p2d", bufs=2, space="PSUM") as p2d:

                MT = pMT.tile([P, ICH, G], bf16, tag="MT")   # [j-part, jc, i]
                for ic in range(ICH):
                    mrow = p2w.tile([P, G], bf16, tag="mrow")
                    nc.sync.dma_start(out=mrow, in_=Md[ic * P:(ic + 1) * P, :])
                    for jh in range(2):
                        pt = p2s.tile([P, D], bf16, tag="mpt")
                        for k in range(4):
                            jc = jh * 4 + k
                            nc.tensor.transpose(pt[:, k * P:(k + 1) * P],
                                                mrow[:, jc * P:(jc + 1) * P], identb)
                        nc.scalar.activation(
                            out=MT[:, jh * 4:(jh + 1) * 4, ic * P:(ic + 1) * P],
                            in_=pt.rearrange("p (c i) -> p c i", i=P),
                            func=AF.Copy)

                for h in range(H):
                    hr = (h % 2) * HD
                    hc = h // 2
                    oa = [p2o.tile([HD + 1, IH], f32, tag="oa", name=f"oa_{h}_{i}") for i in range(NIH)]
                    d1 = [p2d.tile([1, IH], f32, tag="d1", name=f"d1_{h}_{i}") for i in range(NIH)]
                    for jc in range(ICH):
                        et = p2w.tile([P, G], bf16, tag="et")
                        pt_ = p2w.tile([P, G], bf16, tag="pt")
                        for ih in range(NIH):
                            sp = p2s.tile([P, IH], f32, tag="sp")
                            nc.tensor.matmul(
                                sp,
                                KfT[hr:hr + HD, hc, jc * P:(jc + 1) * P],
                                QfT[hr:hr + HD, hc, ih * IH:(ih + 1) * IH],
                                start=True, stop=True)
                            nc.scalar.activation(out=et[:, ih * IH:(ih + 1) * IH],
                                                 in_=sp, func=AF.Exp, scale=SCALE)
                        nc.vector.tensor_tensor(out=pt_, in0=et, in1=MT[:, jc, :],
                                                op=OP.mult)
                        for ih in range(NIH):
                            nc.tensor.matmul(oa[ih],
                                             V_st[:, jc, h, :],
                                             pt_[:, ih * IH:(ih + 1) * IH],
                                             start=(jc == 0), stop=(jc == ICH - 1))
                        for ih in range(NIH):
                            nc.tensor.matmul(d1[ih],
                                             ones_col,
                                             et[:, ih * IH:(ih + 1) * IH],
                                             start=(jc == 0), stop=(jc == ICH - 1))
                    for ih in range(NIH):
                        isl = slice(ih * IH, (ih + 1) * IH)
                        nc.scalar.activation(out=OT[hr:hr + HD, hc, isl],
                                             in_=oa[ih][0:HD, :], func=AF.Copy)
                        # stage single stat rows at base partition 0, then DMA
                        # into the stacked stat tiles (engines need 32-aligned
                        # partition bases; DMA does not)
                        s2st = p2w.tile([1, IH], f32, tag="s2st",
                                        name=f"s2st_{h}_{ih}")
                        nc.scalar.activation(out=s2st, in_=oa[ih][HD:HD + 1, :],
                                             func=AF.Copy)
                        nc.sync.dma_start(out=ST_s2[h:h + 1, isl], in_=s2st)
                        d1st = p2w.tile([1, IH], f32, tag="d1st",
                                        name=f"d1st_{h}_{ih}")
                        nc.scalar.activation(out=d1st, in_=d1[ih], func=AF.Copy)
                        nc.sync.dma_start(out=ST_d1[h:h + 1, isl], in_=d1st)

                # scale_rows = 1 / (cA*s2 + cB*d1)
                t_a = p2w.tile([H, G], f32, tag="t_a")
                nc.vector.tensor_scalar_mul(out=t_a, in0=ST_s2, scalar1=cA_t[0:H])
                t_b = p2w.tile([H, G], f32, tag="t_b")
                nc.vector.tensor_scalar_mul(out=t_b, in0=ST_d1, scalar1=cB_t[0:H])
                nc.vector.tensor_add(out=t_a, in0=t_a, in1=t_b)
                nc.vector.reciprocal(out=scale_rows, in_=t_a)
                nc.vector.tensor_copy(out=scale_bf, in_=scale_rows)

                # apply per-(head, i) scale to OT rows: replicate the scale
                # row across 64 partitions with a k=1 ones-matmul into PSUM
                for h in range(H):
                    hr = (h % 2) * HD
                    hc = h // 2
                    srow = p2w.tile([1, G], bf16, tag="srow", name=f"srow_{h}")
                    nc.sync.dma_start(out=srow, in_=scale_bf[h:h + 1, :])
                    for ih in range(NIH):
                        isl = slice(ih * IH, (ih + 1) * IH)
                        srep = p2s.tile([HD, IH], f32, tag="sp",
                                        name=f"srep_{h}_{ih}")
                        nc.tensor.matmul(srep, ones_row, srow[:, isl],
                                         start=True, stop=True)
                        nc.vector.tensor_tensor(out=OT[hr:hr + HD, hc, isl],
                                                in0=OT[hr:hr + HD, hc, isl],
                                                in1=srep, op=OP.mult)

            # ============ phase 3: out-proj, residual, LN2, FFN ===========
            pB_cm.__exit__(None, None, None)
            with tc.tile_pool(name="p3", bufs=1) as p3, \
                 tc.tile_pool(name="p3w", bufs=3) as p3w, \
                 tc.tile_pool(name="p3ps", bufs=4, space="PSUM") as p3ps, \
                 tc.tile_pool(name="p3pt", bufs=2, space="PSUM") as p3pt:

                wo_t = p3.tile([P, KC, D], bf16, tag="wo_t")
                nc.sync.dma_start(out=wo_t, in_=wo.rearrange("(c p) n -> p c n", p=P))
                w1_t = p3.tile([P, KC, DFF], bf16, tag="w1_t")
                nc.sync.dma_start(out=w1_t, in_=w1.rearrange("(c p) n -> p c n", p=P))
                w2_t = p3.tile([P, FC, D], bf16, tag="w2_t")
                nc.sync.dma_start(out=w2_t, in_=w2.rearrange("(c p) n -> p c n", p=P))
                x2T = p3.tile([P, KC, G], bf16, tag="x2T")

                for ic in range(ICH):
                    ps = p3ps.tile([P, D], f32, tag="ps")
                    for dc in range(KC):
                        nc.tensor.matmul(ps,
                                         OT[:, dc, ic * P:(ic + 1) * P],
                                         wo_t[:, dc, :],
                                         start=(dc == 0), stop=(dc == KC - 1))
                    in2c = p3w.tile([P, D], f32, tag="in2c")
                    nc.sync.dma_start(out=in2c, in_=in2[ic * P:(ic + 1) * P, :])
                    t0 = p3w.tile([P, D], f32, tag="res_t0")
                    nc.vector.tensor_add(out=t0, in0=ps, in1=in2c)
                    nc.gpsimd.tensor_tensor(out=h_res[:, ic, :], in0=t0,
                                            in1=bo_rep, op=OP.add)
                    x2c = p3w.tile([P, D], bf16, tag="x2c")
                    ln_chunk(h_res[:, ic, :], x2c, p3w, g2_rep, b2_rep)
                    pt = p3pt.tile([P, D], bf16, tag="pt")
                    for kc in range(KC):
                        nc.tensor.transpose(pt[:, kc * P:(kc + 1) * P],
                                            x2c[:, kc * P:(kc + 1) * P], identb)
                    nc.scalar.activation(
                        out=x2T[:, :, ic * P:(ic + 1) * P],
                        in_=pt.rearrange("p (c i) -> p c i", i=P),
                        func=AF.Copy)

                h1g = p3.tile([P, FC, G], bf16, tag="h1g")
                o2T = p3.tile([P, KC, G], bf16, tag="o2T")
                for fc in range(FC):
                    pss = [p3ps.tile([P, IH], f32, tag="ps",
                                     name=f"psf_{fc}_{ih}") for ih in range(NIH)]
                    for kc in range(KC):
                        for ih in range(NIH):
                            nc.tensor.matmul(pss[ih],
                                             w1_t[:, kc, fc * P:(fc + 1) * P],
                                             x2T[:, kc, ih * IH:(ih + 1) * IH],
                                             start=(kc == 0), stop=(kc == KC - 1))
                    for ih in range(NIH):
                        nc.scalar.activation(out=h1g[:, fc, ih * IH:(ih + 1) * IH],
                                             in_=pss[ih], func=AF.Gelu,
                                             bias=b1_c[:, fc:fc + 1], scale=1.0)
                for dc in range(KC):
                    pss = [p3ps.tile([P, IH], f32, tag="ps",
                                     name=f"psd_{dc}_{ih}") for ih in range(NIH)]
                    for fc in range(FC):
                        for ih in range(NIH):
                            nc.tensor.matmul(pss[ih],
                                             w2_t[:, fc, dc * P:(dc + 1) * P],
                                             h1g[:, fc, ih * IH:(ih + 1) * IH],
                                             start=(fc == 0), stop=(fc == FC - 1))
                    for ih in range(NIH):
                        nc.scalar.activation(out=o2T[:, dc, ih * IH:(ih + 1) * IH],
                                             in_=pss[ih], func=AF.Identity,
                                             bias=b2_c[:, dc:dc + 1], scale=1.0)
                for ic in range(ICH):
                    pt = p3pt.tile([P, D], bf16, tag="pt")
                    for dc in range(KC):
                        nc.tensor.transpose(pt[:, dc * P:(dc + 1) * P],
                                            o2T[:, dc, ic * P:(ic + 1) * P],
                                            identb)
                    outc = p3w.tile([P, D], f32, tag="outc")
                    nc.vector.tensor_add(out=outc, in0=pt, in1=h_res[:, ic, :])
                    nc.sync.dma_start(out=out_d[ic * P:(ic + 1) * P, :], in_=outc)

    if split_waits:
        _split_sync_waits(nc, mybir)
    return nc


def _split_sync_waits(nc, mybir, maxw=1):
    """walrus CoreV3 codegen allows only one sem wait per instruction; move
    excess waits onto same-engine nops inserted before the instruction."""
    nid = 0
    for fn in nc.m.functions:
        for blk in fn.blocks:
            orig = list(blk.instructions)
            if not any(i.sync_info and i.sync_info.on_wait and
                       len(i.sync_info.on_wait) > maxw for i in orig):
                continue
            new = []
            for ins in orig:
                si = ins.sync_info
                waits = list(si.on_wait) if si and si.on_wait else []
                if len(waits) > maxw:
                    si.on_wait = waits[:maxw]
                    for k in range(maxw, len(waits), maxw):
                        nop = mybir.InstNoOp(name=f"I-wsplit-{nid}", ins=[], outs=[])
                        nid += 1
                        nop.engine = ins.engine
                        nop.sync_info = mybir.SyncInfo(
                            on_wait=waits[k:k + maxw], on_update=[])
                        new.append(nop)
                new.append(ins)
            blk.instructions = new


def _in_maps(inputs):
    import ml_dtypes
    BF = ml_dtypes.bfloat16
    inp = {k: np.ascontiguousarray(np.asarray(v, np.float32)) for k, v in inputs.items()}
    I64 = np.eye(HD, dtype=np.float32)
    sel2 = np.concatenate([np.zeros((HD, HD), np.float32), I64], 0)
    z64 = np.zeros(HD, np.float32)

    def fold_w(w, f):  # [D, D(out by head)] x [HD, HD] block product
        return np.einsum('ihk,kj->ihj', w.reshape(D, H, HD), f).reshape(D, D)

    def fold_b(b1v, b2v, f, fb):  # effective fused bias, feature-ordered [D]
        e = (np.einsum('hk,kj->hj', b1v.reshape(H, HD), f[:HD]) +
             np.einsum('hk,kj->hj', b2v.reshape(H, HD), f[HD:]) + fb[None, :])
        return e.reshape(D)

    maps = []
    for core in range(8):
        b = core % 4
        if core < 4:  # gene branch
            wq1, wq2 = inp["gene_wq"], inp["gene_wq"]
            wk1, wk2 = inp["gene_wk"], inp["gene_wk"]
            bq1 = bq2 = inp["gene_bq"]; bk1 = bk2 = inp["gene_bk"]
            fwq, fbqv, fwk, fbkv = sel2, z64, sel2, z64
            m = dict(
                in1=inp["gene_emb"][b], in2=inp["gene_emb"][b],
                ln1_g=inp["ln_g1_g"], ln1_b=inp["ln_g1_b"],
                ln2_g=inp["ln_g2_g"], ln2_b=inp["ln_g2_b"],
                wv2=inp["gene_wv"], bv2=inp["gene_bv"],
                wo=inp["gout_w"], bo=inp["gout_b"],
                w1=inp["ffn_g_w1"], b1=inp["ffn_g_b1"],
                w2=inp["ffn_g_w2"], b2=inp["ffn_g_b2"],
                coef=np.array([1.0, 1e-8], np.float32),
            )
        else:  # expr branch
            wq1, wq2 = inp["gene_wq"], inp["expr_wq"]
            wk1, wk2 = inp["gene_wk"], inp["expr_wk"]
            bq1, bq2 = inp["gene_bq"], inp["expr_bq"]
            bk1, bk2 = inp["gene_bk"], inp["expr_bk"]
            fwq, fbqv = inp["fused_wq"], inp["fused_bq"]
            fwk, fbkv = inp["fused_wk"], inp["fused_bk"]
            m = dict(
                in1=inp["gene_emb"][b], in2=inp["expr_emb"][b],
                ln1_g=inp["ln_e1_g"], ln1_b=inp["ln_e1_b"],
                ln2_g=inp["ln_e2_g"], ln2_b=inp["ln_e2_b"],
                wv2=inp["expr_wv"], bv2=inp["expr_bv"],
                wo=inp["eout_w"], bo=inp["eout_b"],
                w1=inp["ffn_e_w1"], b1=inp["ffn_e_b1"],
                w2=inp["ffn_e_w2"], b2=inp["ffn_e_b2"],
                coef=np.array([0.0, 1.0], np.float32),
            )
        m["M"] = inp["M"][b].astype(BF)
        m["wcq1"] = fold_w(wq1, fwq[:HD]).astype(BF)
        m["wcq2"] = fold_w(wq2, fwq[HD:]).astype(BF)
        m["wck1"] = fold_w(wk1, fwk[:HD]).astype(BF)
        m["wck2"] = fold_w(wk2, fwk[HD:]).astype(BF)
        m["fbq"] = fold_b(bq1, bq2, fwq, fbqv)
        m["fbk"] = fold_b(bk1, bk2, fwk, fbkv)
        for k in ("wv2", "wo", "w1", "w2"):
            m[k] = m[k].astype(BF)
        maps.append({k: np.ascontiguousarray(v) for k, v in m.items()})
    return maps


def kernel(**inputs):
    from concourse.bass_utils import run_bass_kernel_spmd

    if "nc" not in _cache:
        _cache["nc"] = _build_program()
    nc = _cache["nc"]

    res = run_bass_kernel_spmd(nc, _in_maps(inputs), core_ids=list(range(8)))
    out_gene = np.stack([res.results[c]["out"] for c in range(4)])
    out_expr = np.stack([res.results[c]["out"] for c in range(4, 8)])
    return (out_gene, out_expr)


# revision 10
# speedup vs baseline: 2.9474x; 1.2835x over previous
"""Trainium2 Bass kernel for nn_DeepSCTransformerBlock.

Sharding: 8 cores = 4 batches x 2 branches (gene/expr). Zero collectives.
One SPMD program; the gene branch is expressed through the expr skeleton by
host-side weight folding, and the (s2, d1) normalization coefficients
(cA, cB) are per-core input data:
  gene: scale_i = 1/(1*s2 + 1e-8*d1)   [signed-L1 renorm of softmax*mask]
  expr: scale_i = 1/(0*s2 + 1*d1)      [plain softmax denominator]
where E = exp(S), P = E*M, d1 = colsum(E), s2 = colsum(P); O = scale * (P @ V).

Host-side folds (all exact, fp32):
  - fused head projection: Qf = concat(Q1,Q2) @ fwq + fbq collapses into two
    D x D weights (per-head block product) -> one accumulated projection.
  - LayerNorm affine (gamma/beta) folded into every consumer weight/bias, so
    on-chip LN is just (x - mean) * rstd.
  - out-proj bias bo folded into the residual input (in2b = in2 + bo).
  - M passed pre-transposed ([j, i]) so the mask tile is a straight DMA.

All matmul operands are bf16 (1 PE cycle/row vs 4 for fp32); PSUM accumulates
fp32; the residual stream, LN statistics, and softmax scale math stay fp32.
Activations flow feature-major ([d, tokens]) through the matmul chains; S is
computed transposed ([j, i]) so the AV contraction needs no transposes of the
attention matrix; s2 falls out of a ones-column appended to V; FFN2 contracts
feature-major h1 directly into token-major output (no final transposes).
"""

import numpy as np

B, G, D, H = 4, 1024, 512, 8
HD = D // H
DFF = 4 * D
P = 128
ICH = G // P      # 8 token chunks
KC = D // P       # 4 feature chunks
FC = DFF // P     # 16 ffn-hidden chunks
NIH = 2           # i halves (free-dim 512 per matmul)
IH = G // NIH     # 512
SCALE = 1.0 / (HD ** 0.5)
EPS = 1e-5

_cache = {}


def _build_program(split_waits=True):
    import contextlib
    import concourse.bass as bass
    import concourse.mybir as mybir
    import concourse.tile as tile
    from concourse.masks import make_identity

    # walrus CoreV3 codegen rejects instructions carrying >1 sem wait at the
    # Tile end-of-kernel drain; split the waits across single-wait nops.
    def _patched_drain_and_barrier(self, tick_clock, wait_clock):
        nc = self.nc
        drain_inst = nc.sync.drain()
        wait_clock.add_sem_waits(
            drain_inst.ins, tile.ScopedClock({None: tick_clock.global_clock})
        )
        si = drain_inst.ins.sync_info
        if si is not None and si.on_wait and len(si.on_wait) > 1:
            waits = list(si.on_wait)
            si.on_wait = waits[:1]
            for i in range(1, len(waits)):
                nop = nc.sync.nop(hint="drain_wait_split", nofuse=True)
                nop.ins.sync_info = mybir.SyncInfo(
                    on_wait=waits[i : i + 1], on_update=[]
                )
        nc.all_engine_barrier()
        assert self.sems is not None
        popped = nc._tile_sem_poison_stack.pop()
        assert popped is self._sem_poison
        nc.clear_and_free_semaphores(list(self.sems.allocated().values()))
        nc.all_engine_barrier()

    tile.TileContext._drain_and_barrier = _patched_drain_and_barrier

    f32 = mybir.dt.float32
    bf16 = mybir.dt.bfloat16
    AF = mybir.ActivationFunctionType
    OP = mybir.AluOpType

    nc = bass.Bass()

    def dram_in(name, shape, dt=f32):
        return nc.dram_tensor(name, list(shape), dt, kind="ExternalInput")

    in1 = dram_in("in1", (G, D))
    in2 = dram_in("in2", (G, D))
    in2b = dram_in("in2b", (G, D))           # in2 + bo (residual base)
    MdT = dram_in("MT", (G, G), bf16)        # pre-transposed mask [j, i]
    wcq1 = dram_in("wcq1", (D, D), bf16)
    wcq2 = dram_in("wcq2", (D, D), bf16)
    wck1 = dram_in("wck1", (D, D), bf16)
    wck2 = dram_in("wck2", (D, D), bf16)
    fbq = dram_in("fbq", (D,))
    fbk = dram_in("fbk", (D,))
    wv2 = dram_in("wv2", (D, D), bf16); bv2 = dram_in("bv2", (D,))
    wo = dram_in("wo", (D, D), bf16)
    w1 = dram_in("w1", (D, DFF), bf16); b1 = dram_in("b1", (DFF,))
    w2 = dram_in("w2", (DFF, D), bf16); b2 = dram_in("b2", (D,))
    coef = dram_in("coef", (2,))

    out_d = nc.dram_tensor("out", [G, D], f32, kind="ExternalOutput")

    with tile.TileContext(nc) as tc:
        with contextlib.ExitStack() as ctx:
            pc = ctx.enter_context(tc.tile_pool(name="const", bufs=1))
            identb = pc.tile([P, P], bf16, tag="identb")
            make_identity(nc, identb)
            eps_t = pc.tile([P, 1], f32, tag="eps")
            nc.vector.memset(eps_t, EPS)
            cA_t = pc.tile([P, 1], f32, tag="cA")
            nc.sync.dma_start(out=cA_t, in_=coef[None, 0:1].to_broadcast([P, 1]))
            cB_t = pc.tile([P, 1], f32, tag="cB")
            nc.sync.dma_start(out=cB_t, in_=coef[None, 1:2].to_broadcast([P, 1]))

            def rep_row(name, vec, n=D):  # [n] dram -> [P, n] replicated rows
                t = pc.tile([P, n], f32, tag=name)
                nc.sync.dma_start(out=t, in_=vec[None, :].to_broadcast([P, n]))
                return t

            bv2_rep = rep_row("bv2_rep", bv2)
            b2_rep = rep_row("b2_rep", b2)

            def col(name, vec, n):  # [n*P] dram -> [P, n] column tile
                t = pc.tile([P, n], f32, tag=name)
                nc.sync.dma_start(out=t, in_=vec.rearrange("(c p) -> p c", p=P))
                return t

            fbq_c = col("fbq_c", fbq, KC); fbk_c = col("fbk_c", fbk, KC)
            b1_c = col("b1_c", b1, FC)
            ones_row = pc.tile([1, HD], bf16, tag="ones_row")
            nc.vector.memset(ones_row, 1.0)

            # long-lived state: residual stream, stats, mask, late weights
            pD = ctx.enter_context(tc.tile_pool(name="resid", bufs=1))
            OT = pD.tile([P, KC, G], bf16, tag="OT")        # merged heads, feature-major
            h_res = pD.tile([P, ICH, D], f32, tag="h_res")  # token-major residual
            ST_d1 = pD.tile([H, G], f32, tag="ST_d1")       # row h: d1_h
            ST_s2 = pD.tile([H, G], f32, tag="ST_s2")       # row h: s2_h
            scale_rows = pD.tile([H, G], f32, tag="scale_rows")
            scale_bf = pD.tile([H, G], bf16, tag="scale_bf")
            MT = pD.tile([P, ICH, G], bf16, tag="MT")       # [j-part, jc, i]
            nc.sync.dma_start(out=MT,
                              in_=MdT.rearrange("(jc p) i -> p jc i", p=P))
            # prefetch late-phase weights while phase 1/2 compute
            wo_t = pD.tile([P, KC, D], bf16, tag="wo_t")
            nc.sync.dma_start(out=wo_t, in_=wo.rearrange("(c p) n -> p c n", p=P))
            w1_t = pD.tile([P, KC, DFF], bf16, tag="w1_t")
            nc.sync.dma_start(out=w1_t, in_=w1.rearrange("(c p) n -> p c n", p=P))
            w2_t = pD.tile([P, FC, D], bf16, tag="w2_t")
            nc.sync.dma_start(out=w2_t, in_=w2.rearrange("(c p) n -> p c n", p=P))

            # long-lived attention operands (packed head layout: head h ->
            # partition rows (h%2)*64, chunk h//2); freed before phase 3
            pB_cm = tc.tile_pool(name="attn_ops", bufs=1)
            pB = pB_cm.__enter__()
            QfT = pB.tile([P, H // 2, G], bf16, tag="QfT")
            KfT = pB.tile([P, H // 2, G], bf16, tag="KfT")
            V_st = pB.tile([P, ICH, H, HD + 1], bf16, tag="V_st")  # [j, jc, h, 65]

            ones_col = V_st[:, 0, 0, HD:HD + 1]   # [128, 1] of ones
            nc.vector.memset(V_st, 1.0)           # ones cols; V parts overwritten

            def ln_chunk(src_ap, xg_out, wkp):
                """Plain LayerNorm (no affine) of a [P, D] fp32 chunk -> bf16."""
                stats = wkp.tile([P, 6], f32, tag="ln_stats")
                mv = wkp.tile([P, 2], f32, tag="ln_mv")
                nc.vector.bn_stats(out=stats, in_=src_ap)
                nc.vector.bn_aggr(out=mv, in_=stats)
                stdt = wkp.tile([P, 1], f32, tag="ln_std")
                nc.scalar.activation(out=stdt, in_=mv[:, 1:2], func=AF.Sqrt,
                                     bias=eps_t, scale=1.0)
                rstd = wkp.tile([P, 1], f32, tag="ln_rstd")
                nc.vector.reciprocal(out=rstd, in_=stdt)
                nc.vector.tensor_scalar(out=xg_out, in0=src_ap, scalar1=mv[:, 0:1],
                                        scalar2=rstd, op0=OP.subtract, op1=OP.mult)

            # ============ phase 1: LN1, V proj, folded Qf/Kf proj ==========
            with tc.tile_pool(name="p1", bufs=1) as p1, \
                 tc.tile_pool(name="p1w", bufs=2) as p1w, \
                 tc.tile_pool(name="p1ps", bufs=2, space="PSUM") as p1ps, \
                 tc.tile_pool(name="p1qk", bufs=2, space="PSUM") as p1qk, \
                 tc.tile_pool(name="p1pt", bufs=2, space="PSUM") as p1pt:

                xgeT = p1.tile([P, KC, G], bf16, tag="xgeT")
                xeeT = p1.tile([P, KC, G], bf16, tag="xeeT")
                for (src, dstT) in ((in1, xgeT), (in2, xeeT)):
                    for ic in range(ICH):
                        xc = p1w.tile([P, D], f32, tag="ln_in")
                        nc.sync.dma_start(out=xc, in_=src[ic * P:(ic + 1) * P, :])
                        xg = p1w.tile([P, D], bf16, tag="ln_out")
                        ln_chunk(xc, xg, p1w)
                        pt = p1pt.tile([P, D], bf16, tag="pt")
                        for kc in range(KC):
                            nc.tensor.transpose(pt[:, kc * P:(kc + 1) * P],
                                                xg[:, kc * P:(kc + 1) * P], identb)
                        nc.scalar.activation(
                            out=dstT[:, :, ic * P:(ic + 1) * P],
                            in_=pt.rearrange("p (c i) -> p c i", i=P),
                            func=AF.Copy)

                def load_w(name, wd):
                    t = p1.tile([P, KC, D], bf16, tag=name)
                    nc.sync.dma_start(out=t, in_=wd.rearrange("(c p) n -> p c n", p=P))
                    return t

                wcq1_t = load_w("wcq1_t", wcq1)
                wcq2_t = load_w("wcq2_t", wcq2)
                wck1_t = load_w("wck1_t", wck1)
                wck2_t = load_w("wck2_t", wck2)
                wv2_t = load_w("wv2_t", wv2)

                # V projection (token-major) into V_st slots
                for jc in range(ICH):
                    ps = p1ps.tile([P, D], f32, tag="ps")
                    for kc in range(KC):
                        nc.tensor.matmul(ps,
                                         xeeT[:, kc, jc * P:(jc + 1) * P],
                                         wv2_t[:, kc, :],
                                         start=(kc == 0), stop=(kc == KC - 1))
                    nc.vector.tensor_tensor(
                        out=V_st[:, jc, :, 0:HD],
                        in0=ps.rearrange("p (h d) -> p h d", d=HD),
                        in1=bv2_rep.rearrange("p (h d) -> p h d", d=HD),
                        op=OP.add)

                # folded Qf/Kf projections, per head-pair chunk c:
                # QfT[:,c] = Wcq1[:,c]^T @ xgeT + Wcq2[:,c]^T @ xeeT + fbq
                for c in range(KC):
                    csl = slice(c * P, (c + 1) * P)
                    for (wt1, wt2, bias_c, dstT) in (
                        (wcq1_t, wcq2_t, fbq_c, QfT),
                        (wck1_t, wck2_t, fbk_c, KfT),
                    ):
                        pss = p1qk.tile([P, G], f32, tag="psqk",
                                        name=f"psqk_{c}_{dstT is KfT}")
                        for si, (wt, srcT) in enumerate(((wt1, xgeT), (wt2, xeeT))):
                            for kc in range(KC):
                                st = (si == 0 and kc == 0)
                                sp_ = (si == 1 and kc == KC - 1)
                                for ih in range(NIH):
                                    nc.tensor.matmul(
                                        pss[:, ih * IH:(ih + 1) * IH],
                                        wt[:, kc, csl],
                                        srcT[:, kc, ih * IH:(ih + 1) * IH],
                                        start=st, stop=sp_)
                        nc.scalar.activation(
                            out=dstT[:, c, :], in_=pss, func=AF.Identity,
                            bias=bias_c[:, c:c + 1], scale=1.0)

            # ================= phase 2: attention =========================
            with tc.tile_pool(name="p2w", bufs=2) as p2w, \
                 tc.tile_pool(name="p2s", bufs=2, space="PSUM") as p2s, \
                 tc.tile_pool(name="p2o", bufs=2, space="PSUM") as p2o, \
                 tc.tile_pool(name="p2d", bufs=2, space="PSUM") as p2d:

                for h in range(H):
                    hr = (h % 2) * HD
                    hc = h // 2
                    oa = [p2o.tile([HD + 1, IH], f32, tag="oa", name=f"oa_{h}_{i}") for i in range(NIH)]
                    d1 = [p2d.tile([1, IH], f32, tag="d1", name=f"d1_{h}_{i}") for i in range(NIH)]
                    for jc in range(ICH):
                        et = p2w.tile([P, G], bf16, tag="et")
                        pt_ = p2w.tile([P, G], bf16, tag="pt")
                        sp = p2s.tile([P, G], f32, tag="sp")
                        for ih in range(NIH):
                            nc.tensor.matmul(
                                sp[:, ih * IH:(ih + 1) * IH],
                                KfT[hr:hr + HD, hc, jc * P:(jc + 1) * P],
                                QfT[hr:hr + HD, hc, ih * IH:(ih + 1) * IH],
                                start=True, stop=True)
                        nc.scalar.activation(out=et, in_=sp, func=AF.Exp,
                                             scale=SCALE)
                        nc.vector.tensor_tensor(out=pt_, in0=et, in1=MT[:, jc, :],
                                                op=OP.mult)
                        for ih in range(NIH):
                            nc.tensor.matmul(oa[ih],
                                             V_st[:, jc, h, :],
                                             pt_[:, ih * IH:(ih + 1) * IH],
                                             start=(jc == 0), stop=(jc == ICH - 1))
                        for ih in range(NIH):
                            nc.tensor.matmul(d1[ih],
                                             ones_col,
                                             et[:, ih * IH:(ih + 1) * IH],
                                             start=(jc == 0), stop=(jc == ICH - 1))
                    for ih in range(NIH):
                        isl = slice(ih * IH, (ih + 1) * IH)
                        nc.vector.tensor_copy(out=OT[hr:hr + HD, hc, isl],
                                              in_=oa[ih][0:HD, :])
                        # stage single stat rows at base partition 0, then DMA
                        # into the stacked stat tiles (engines need 32-aligned
                        # partition bases; DMA does not)
                        s2st = p2w.tile([1, IH], f32, tag="s2st",
                                        name=f"s2st_{h}_{ih}")
                        nc.vector.tensor_copy(out=s2st, in_=oa[ih][HD:HD + 1, :])
                        nc.sync.dma_start(out=ST_s2[h:h + 1, isl], in_=s2st)
                        d1st = p2w.tile([1, IH], f32, tag="d1st",
                                        name=f"d1st_{h}_{ih}")
                        nc.vector.tensor_copy(out=d1st, in_=d1[ih])
                        nc.sync.dma_start(out=ST_d1[h:h + 1, isl], in_=d1st)

                # scale_rows = 1 / (cA*s2 + cB*d1)
                t_a = p2w.tile([H, G], f32, tag="t_a")
                nc.vector.tensor_scalar_mul(out=t_a, in0=ST_s2, scalar1=cA_t[0:H])
                t_b = p2w.tile([H, G], f32, tag="t_b")
                nc.vector.tensor_scalar_mul(out=t_b, in0=ST_d1, scalar1=cB_t[0:H])
                nc.vector.tensor_add(out=t_a, in0=t_a, in1=t_b)
                nc.vector.reciprocal(out=scale_rows, in_=t_a)
                nc.vector.tensor_copy(out=scale_bf, in_=scale_rows)

                # apply per-(head, i) scale to OT rows: replicate the scale
                # row across 64 partitions with a k=1 ones-matmul into PSUM
                for h in range(H):
                    hr = (h % 2) * HD
                    hc = h // 2
                    srow = p2w.tile([1, G], bf16, tag="srow", name=f"srow_{h}")
                    nc.sync.dma_start(out=srow, in_=scale_bf[h:h + 1, :])
                    for ih in range(NIH):
                        isl = slice(ih * IH, (ih + 1) * IH)
                        srt = p2s.tile([P, G], f32, tag="sp",
                                       name=f"srep_{h}_{ih}")
                        srep = srt[0:HD, 0:IH]
                        nc.tensor.matmul(srep, ones_row, srow[:, isl],
                                         start=True, stop=True)
                        nc.vector.tensor_tensor(out=OT[hr:hr + HD, hc, isl],
                                                in0=OT[hr:hr + HD, hc, isl],
                                                in1=srep, op=OP.mult)

            # ============ phase 3: out-proj, residual, LN2, FFN ===========
            pB_cm.__exit__(None, None, None)
            with tc.tile_pool(name="p3", bufs=1) as p3, \
                 tc.tile_pool(name="p3w", bufs=3) as p3w, \
                 tc.tile_pool(name="p3ps", bufs=2, space="PSUM") as p3ps, \
                 tc.tile_pool(name="p3pf", bufs=2, space="PSUM") as p3pf, \
                 tc.tile_pool(name="p3pt", bufs=2, space="PSUM") as p3pt:

                x2T = p3.tile([P, KC, G], bf16, tag="x2T")

                for ic in range(ICH):
                    ps = p3ps.tile([P, D], f32, tag="ps")
                    for dc in range(KC):
                        nc.tensor.matmul(ps,
                                         OT[:, dc, ic * P:(ic + 1) * P],
                                         wo_t[:, dc, :],
                                         start=(dc == 0), stop=(dc == KC - 1))
                    in2c = p3w.tile([P, D], f32, tag="in2c")
                    nc.sync.dma_start(out=in2c, in_=in2b[ic * P:(ic + 1) * P, :])
                    nc.vector.tensor_add(out=h_res[:, ic, :], in0=ps, in1=in2c)
                    x2c = p3w.tile([P, D], bf16, tag="x2c")
                    ln_chunk(h_res[:, ic, :], x2c, p3w)
                    pt = p3pt.tile([P, D], bf16, tag="pt")
                    for kc in range(KC):
                        nc.tensor.transpose(pt[:, kc * P:(kc + 1) * P],
                                            x2c[:, kc * P:(kc + 1) * P], identb)
                    nc.scalar.activation(
                        out=x2T[:, :, ic * P:(ic + 1) * P],
                        in_=pt.rearrange("p (c i) -> p c i", i=P),
                        func=AF.Copy)

                h1g = p3.tile([P, FC, G], bf16, tag="h1g")
                for fc in range(FC):
                    pss = p3pf.tile([P, G], f32, tag="psf", name=f"psf_{fc}")
                    for kc in range(KC):
                        for ih in range(NIH):
                            nc.tensor.matmul(pss[:, ih * IH:(ih + 1) * IH],
                                             w1_t[:, kc, fc * P:(fc + 1) * P],
                                             x2T[:, kc, ih * IH:(ih + 1) * IH],
                                             start=(kc == 0), stop=(kc == KC - 1))
                    nc.scalar.activation(out=h1g[:, fc, :], in_=pss, func=AF.Gelu,
                                         bias=b1_c[:, fc:fc + 1], scale=1.0)
                # FFN2 token-major: out[i, d] += h1^T, then + h_res + b2
                for ic in range(ICH):
                    ps = p3ps.tile([P, D], f32, tag="ps", name=f"ps2_{ic}")
                    for fc in range(FC):
                        nc.tensor.matmul(ps,
                                         h1g[:, fc, ic * P:(ic + 1) * P],
                                         w2_t[:, fc, :],
                                         start=(fc == 0), stop=(fc == FC - 1))
                    t0 = p3w.tile([P, D], f32, tag="res_t0")
                    nc.vector.tensor_add(out=t0, in0=ps, in1=h_res[:, ic, :])
                    outc = p3w.tile([P, D], f32, tag="outc")
                    nc.gpsimd.tensor_tensor(out=outc, in0=t0, in1=b2_rep,
                                            op=OP.add)
                    nc.sync.dma_start(out=out_d[ic * P:(ic + 1) * P, :], in_=outc)

    if split_waits:
        _split_sync_waits(nc, mybir)
    return nc


def _split_sync_waits(nc, mybir, maxw=1):
    """walrus CoreV3 codegen allows only one sem wait per instruction; move
    excess waits onto same-engine nops inserted before the instruction."""
    nid = 0
    for fn in nc.m.functions:
        for blk in fn.blocks:
            orig = list(blk.instructions)
            if not any(i.sync_info and i.sync_info.on_wait and
                       len(i.sync_info.on_wait) > maxw for i in orig):
                continue
            new = []
            for ins in orig:
                si = ins.sync_info
                waits = list(si.on_wait) if si and si.on_wait else []
                if len(waits) > maxw:
                    si.on_wait = waits[:maxw]
                    for k in range(maxw, len(waits), maxw):
                        nop = mybir.InstNoOp(name=f"I-wsplit-{nid}", ins=[], outs=[])
                        nid += 1
                        nop.engine = ins.engine
                        nop.sync_info = mybir.SyncInfo(
                            on_wait=waits[k:k + maxw], on_update=[])
                        new.append(nop)
                new.append(ins)
            blk.instructions = new


def _in_maps(inputs):
    import ml_dtypes
    BF = ml_dtypes.bfloat16
    inp = {k: np.ascontiguousarray(np.asarray(v, np.float32)) for k, v in inputs.items()}
    I64 = np.eye(HD, dtype=np.float32)
    sel2 = np.concatenate([np.zeros((HD, HD), np.float32), I64], 0)
    z64 = np.zeros(HD, np.float32)

    def fold_w(w, f):  # [D, D(out by head)] x [HD, HD] block product
        return np.einsum('ihk,kj->ihj', w.reshape(D, H, HD), f).reshape(D, D)

    def fold_b(b1v, b2v, f, fb):  # fused bias of the head projection [D]
        e = (np.einsum('hk,kj->hj', b1v.reshape(H, HD), f[:HD]) +
             np.einsum('hk,kj->hj', b2v.reshape(H, HD), f[HD:]) + fb[None, :])
        return e.reshape(D)

    maps = []
    for core in range(8):
        b = core % 4
        if core < 4:  # gene branch
            wq1, wq2 = inp["gene_wq"], inp["gene_wq"]
            wk1, wk2 = inp["gene_wk"], inp["gene_wk"]
            bq1 = bq2 = inp["gene_bq"]; bk1 = bk2 = inp["gene_bk"]
            fwq, fbqv, fwk, fbkv = sel2, z64, sel2, z64
            ln1g, ln1b = inp["ln_g1_g"], inp["ln_g1_b"]
            ln2g, ln2b = inp["ln_g2_g"], inp["ln_g2_b"]
            m = dict(
                in1=inp["gene_emb"][b], in2=inp["gene_emb"][b],
                in2b=inp["gene_emb"][b] + inp["gout_b"][None, :],
                wv2=inp["gene_wv"], bv2=inp["gene_bv"],
                wo=inp["gout_w"],
                w1=inp["ffn_g_w1"], b1=inp["ffn_g_b1"],
                w2=inp["ffn_g_w2"], b2=inp["ffn_g_b2"],
                coef=np.array([1.0, 1e-8], np.float32),
            )
        else:  # expr branch
            wq1, wq2 = inp["gene_wq"], inp["expr_wq"]
            wk1, wk2 = inp["gene_wk"], inp["expr_wk"]
            bq1, bq2 = inp["gene_bq"], inp["expr_bq"]
            bk1, bk2 = inp["gene_bk"], inp["expr_bk"]
            fwq, fbqv = inp["fused_wq"], inp["fused_bq"]
            fwk, fbkv = inp["fused_wk"], inp["fused_bk"]
            ln1g, ln1b = inp["ln_e1_g"], inp["ln_e1_b"]
            ln2g, ln2b = inp["ln_e2_g"], inp["ln_e2_b"]
            m = dict(
                in1=inp["gene_emb"][b], in2=inp["expr_emb"][b],
                in2b=inp["expr_emb"][b] + inp["eout_b"][None, :],
                wv2=inp["expr_wv"], bv2=inp["expr_bv"],
                wo=inp["eout_w"],
                w1=inp["ffn_e_w1"], b1=inp["ffn_e_b1"],
                w2=inp["ffn_e_w2"], b2=inp["ffn_e_b2"],
                coef=np.array([0.0, 1.0], np.float32),
            )
        m["MT"] = inp["M"][b].T.astype(BF)
        # fold fused head projection, then LN1 affine into the Q/K weights
        wcq1 = fold_w(wq1, fwq[:HD]); wcq2 = fold_w(wq2, fwq[HD:])
        wck1 = fold_w(wk1, fwk[:HD]); wck2 = fold_w(wk2, fwk[HD:])
        m["fbq"] = (fold_b(bq1, bq2, fwq, fbqv)
                    + ln1b @ wcq1 + ln1b @ wcq2)
        m["fbk"] = (fold_b(bk1, bk2, fwk, fbkv)
                    + ln1b @ wck1 + ln1b @ wck2)
        m["wcq1"] = (ln1g[:, None] * wcq1).astype(BF)
        m["wcq2"] = (ln1g[:, None] * wcq2).astype(BF)
        m["wck1"] = (ln1g[:, None] * wck1).astype(BF)
        m["wck2"] = (ln1g[:, None] * wck2).astype(BF)
        # LN1 affine into V projection
        m["bv2"] = m["bv2"] + ln1b @ m["wv2"]
        m["wv2"] = (ln1g[:, None] * m["wv2"]).astype(BF)
        # LN2 affine into FFN first layer
        m["b1"] = m["b1"] + ln2b @ m["w1"]
        m["w1"] = (ln2g[:, None] * m["w1"]).astype(BF)
        for k in ("wo", "w2"):
            m[k] = m[k].astype(BF)
        maps.append({k: np.ascontiguousarray(v) for k, v in m.items()})
    return maps


def kernel(**inputs):
    from concourse.bass_utils import run_bass_kernel_spmd

    if "nc" not in _cache:
        _cache["nc"] = _build_program()
    nc = _cache["nc"]

    res = run_bass_kernel_spmd(nc, _in_maps(inputs), core_ids=list(range(8)))
    out_gene = np.stack([res.results[c]["out"] for c in range(4)])
    out_expr = np.stack([res.results[c]["out"] for c in range(4, 8)])
    return (out_gene, out_expr)
